# revision 8
# baseline (speedup 1.0000x reference)
"""Trainium2 Bass kernel for nn_ActorNetwork (moe_routing).

Design (host-routed expert parallelism, zero collectives):
  reference semantics: with perm = stable argsort(idx),
    h_f[i] = relu(relu(state[perm[i]] @ W1[g(i)] + b1[g(i)]) @ W2 + b2)
    out[i] = tanh(h_f[i] @ W3[idx[i]] + b3[idx[i]])
  where g(i) (the W1 expert of sorted-position i) depends only on which
  sorted-count block position i falls into.  Core c takes exactly the sorted
  block of game c -> its layer-1 is ONE dense matmul with only W1[c].  Within
  the core, rows are sub-grouped by head game idx[i] so layer-3 is 8 dense
  per-group matmuls.  All routing (gather of state rows in, scatter of output
  rows back) happens on the host during shard/unshard.  Groups are stored
  sorted by size (descending); slot capacities M_j = max_core(j-th largest
  group) — sorted-desc is optimal for sum-of-column-maxes (N=1069, ~3% pad).

  On-device layout is feature-major: activations live as [feature, row] so
  every matmul is lhsT=weight-tile [K=128, M=128], rhs=activation [K=128,
  N=rows], PSUM out [M features, rows].  Compute dtype fp16 (more mantissa
  than bf16 at the same PE rate — frees error budget for fp8), f32 PSUM.

Performance structure (measured at the 2.3-2.4GHz PE state):
  - L2's first two k-tiles (K=256 of 1024) run as ONE fp8 e4m3 DoubleRow
    matmul (2 k-tiles per N-cycle pass) for ALL rows; the small tail chunk
    (cols 1024..N) runs its ENTIRE L2 as 4 DoubleRows (its h1 m2-7 stored
    e4m3 at scale 1), cutting its issue-bound pass count 40->32.
    All PSUM partials share one product scale 2^12: h1q x16 with w2q k0-1
    x256, h1q2 x1 with w2q k2-7 x4096, fp16 w2 x2^12, b2 x2^12; W3 ships
    UNSCALED fp16 and the tanh ACT descales via scale=2^-12 (fp16 can't
    hold W3/2^12 — 5-bit exponent underflows).  Measured rel err 0.0188
    of the 2e-2 budget (sim-validated to 1e-5 against HW).
  - Head: the exec window opens at the framework const-memsets (~5.9us,
    fixed) and engine DMA desc-gen can't start before engine boot (~7.2us).
    First pieces are split ACROSS the two HWDGE rings (sync: st_c0 k-tiles;
    scalar: w1 k-tile m0-3 pieces) so the first L1 wave's data lands ~10us;
    44 zero-data warmup matmuls (~2.3us) ramp the PE p-state meanwhile.
  - PE stream is gapless mid-kernel: relu work alternates vector/scalar by
    w1-image position parity; 6-deep PSUM pool; L1 waves chunk-outer
    tracking DMA arrival.
  - Tail: the boundary-crossing group's L3 is split at col 1024 so after
    the tail-chunk L2 only a ~45-col L3+tanh+DMA remains before the exit
    barriers.  The NEFF fini (full 256-sem reset storm, ~7.2us) and boot
    are fixed costs inside the measured window.
  NOTE: the shared trn2 pool's PE clock wanders run to run; absolute times
  scale ~1.2x with it.
"""

import numpy as np
import ml_dtypes

_FP16 = np.float16
_FP8 = ml_dtypes.float8_e4m3     # TRN fp8e4: IEEE-ish e4m3, max normal 240
_NCORES = 8
_SH = 16.0                       # h1 fp8 scale (m0-1)
_SW0 = 256.0                     # w2 k0-1 fp8 scale (16*256 = 2^12)
_SW1 = 4096.0                    # w2 k2-7 fp8 scale (h1q2 at scale 1)
_SF = _SH * _SW0                 # 2^12: scale of all L2 partials / hf
# w1 image position -> logical m.  Relu engine alternates by position
# parity (even -> vector, odd -> scalar) so every L1 wave splits its 4
# relus across both engines; logical m0-1 (fp8, scalar ACT) sit at odd
# positions.
_MPERM = (2, 0, 3, 1, 4, 6, 5, 7)
_WARM_MMS = 44  # PE warmup matmuls: end ~when the first DMA piece lands
_graph_cache: dict = {}


def _make_plan(idx: np.ndarray, G: int):
    """Host routing plan: which (sorted-position) rows go to which core/slot."""
    idx = np.asarray(idx)
    perm = np.argsort(idx, kind="stable")
    counts = np.bincount(idx, minlength=G)
    cum = np.zeros(G + 1, dtype=np.int64)
    cum[1:] = np.cumsum(counts)

    core_groups = []  # per core: list of (head_game, sorted_positions) desc by size
    for c in range(G):
        pos = np.arange(cum[c], cum[c + 1])
        heads = idx[pos]
        groups = [(b, pos[heads == b]) for b in range(G)]
        groups.sort(key=lambda t: (-len(t[1]), t[0]))
        core_groups.append(groups)

    sizes = np.array([[len(p) for _, p in groups] for groups in core_groups])
    M = sizes.max(axis=0)          # slot capacity per position (SPMD-uniform)
    keep = M > 0
    M = M[keep]
    core_groups = [[g for g, k in zip(groups, keep) if k] for groups in core_groups]
    NG = len(M)
    starts = np.zeros(NG + 1, dtype=np.int64)
    starts[1:] = np.cumsum(M)
    N = int(starts[-1])
    return perm, core_groups, M, starts, N


def _chunks_of(N):
    # 512-wide chunks: matmuls with 512 moving cols fully hide LDWEIGHTS.
    out = []
    c0 = 0
    while c0 < N:
        cw = min(512, N - c0)
        out.append((c0, cw))
        c0 += cw
    return out


def _build_graph(D, H1, H2, A, NG, starts, N):
    """Build + finalize the SPMD Bass graph (identical for all cores)."""
    from concourse import bacc
    import concourse.mybir as mybir
    from concourse.tile import TileContext

    f16 = mybir.dt.float16
    f32 = mybir.dt.float32
    fp8 = mybir.dt.float8e4
    KD, K1, K2 = D // 128, H1 // 128, H2 // 128
    M1, M2 = H1 // 128, H2 // 128
    assert D % 128 == 0 and H1 % 128 == 0 and H2 % 128 == 0 and A == 128

    chunks = _chunks_of(N)
    c0w0 = chunks[0][1]
    assert c0w0 == 512
    # fp16 input image (single SBUF tile, range-tracked), laid out so the
    # critical first-wave pieces ride BOTH HWDGE rings in parallel:
    #   [ st_c0 k0..k3 | w1 halfA (k0..k3 x m0-3) | w1 halfB (m4-7)
    #     | st chunks 1.. (k-major per chunk) | w2 k2-7 | w3 slots ]
    A1 = KD * c0w0                      # w1 half A base (positions 0-3)
    A2 = A1 + KD * 512                  # w1 half B base (positions 4-7)
    P0 = A2 + KD * 512                  # st chunks 1..
    P1 = P0 + KD * (N - c0w0)           # w2 fp16 k2-7
    P2 = P1 + (K1 - 2) * H2             # w3 slots
    IMG_W = P2 + NG * K2 * A

    def st_col(k, c0, cw):
        if c0 == 0:
            return k * c0w0
        return P0 + (c0 - c0w0) * KD + k * cw

    def w1_col(k, p):
        if p < 4:
            return A1 + k * 512 + p * 128
        return A2 + k * 512 + (p - 4) * 128

    nc = bacc.Bacc("TRN2")
    img_ext = nc.declare_dram_parameter("img", [128, IMG_W], f16, isOutput=False)
    w2q_ext = nc.declare_dram_parameter("w2q", [128, K1 * H2], fp8, isOutput=False)
    b_ext = nc.declare_dram_parameter("bs", [128, M1 + M2 + NG], f32, isOutput=False)
    out_ext = nc.declare_dram_parameter("out", [A, N], f32, isOutput=True)

    add = mybir.AluOpType.add
    amax = mybir.AluOpType.max
    Tanh = mybir.ActivationFunctionType.Tanh
    Relu = mybir.ActivationFunctionType.Relu

    with TileContext(nc) as tc:
        with (
            tc.tile_pool(name="weights", bufs=1) as wp,
            tc.tile_pool(name="acts", bufs=1) as ap,
            tc.tile_pool(name="psum", bufs=6, space="PSUM") as pp,
            tc.tile_pool(name="psum3", bufs=2, space="PSUM") as pp3,
        ):
            img = wp.tile([128, IMG_W], f16, name="img", tag="img")
            w2qt = wp.tile([128, K1, H2], fp8, name="w2qt", tag="w2qt")
            bt = wp.tile([128, M1 + M2 + NG], f32, name="bt", tag="bt")
            h1q = ap.tile([128, 2, N], fp8, name="h1q", tag="h1q")
            c2w = chunks[-1][1] if chunks[-1][1] < 512 else 0
            # the tail chunk runs L2 k2-7 in fp8 too (3 more DoubleRows):
            # its h1 m2-7 lives here as e4m3 at scale 1
            h1q2 = ap.tile([128, 6, c2w], fp8, name="h1q2", tag="h1q2") if c2w else None
            h1 = [ap.tile([128, N], f16, name=f"h1_{m}", tag=f"h1_{m}") if m >= 2 else None
                  for m in range(M1)]
            hf = [ap.tile([128, N], f16, name=f"hf_{m}", tag=f"hf_{m}") for m in range(M2)]
            osb = ap.tile([A, N], f32, name="osb", tag="osb")
            wrm = wp.tile([128, 64], f16, name="wrm", tag="wrm")

            # PE warmup: ramp the HAM clock gate while the first DMAs land.
            nc.gpsimd.memset(wrm[:], 0)
            pw = pp3.tile([64, 64], f32, name="psw", tag="ps3")
            for _ in range(_WARM_MMS):
                nc.tensor.matmul(pw[:], wrm[:, :64], wrm[:], start=True, stop=True)

            # DMAs: desc-gen is ~0.67us per dma_start, SERIAL per engine, and
            # the first piece can't start before engine boot (~7.2us).  The
            # first L1 k-wave needs st_c0 k0 AND w1 k0 m0-3 — alternate st/w1
            # k-pieces ACROSS the two rings so every k-wave's pair lands in
            # parallel, ~0.67us apart (matching the 0.87us/wave burn rate).
            # Scalar's desc burst must end by ~12.6us (it runs the odd-
            # position relu1 ACTs from ~13.2), so the big late pieces ride
            # sync, which has no elementwise duties.  Completion is per
            # dma_start, so pieces stay <=512KB to keep waits fine-grained.
            def icol(a, b):
                return img[:, a:b], img_ext[:, a:b]

            half2 = P1 + ((K1 - 2) // 2) * H2
            nc.sync.dma_start(*icol(0, c0w0))                        # st_c0 k0
            nc.scalar.dma_start(*icol(A1, A1 + 512))                 # w1A k0
            nc.sync.dma_start(*icol(A1 + 512, A1 + 1024))            # w1A k1
            nc.scalar.dma_start(*icol(c0w0, 2 * c0w0))               # st_c0 k1
            nc.sync.dma_start(*icol(2 * c0w0, 3 * c0w0))             # st_c0 k2
            nc.scalar.dma_start(*icol(A1 + 1024, A1 + 1536))         # w1A k2
            nc.sync.dma_start(*icol(A1 + 1536, A1 + 2048))           # w1A k3
            nc.scalar.dma_start(*icol(3 * c0w0, 4 * c0w0))           # st_c0 k3
            nc.sync.dma_start(bt[:], b_ext[:])                       # bias (~13.2us)
            nc.scalar.dma_start(*icol(A2, A2 + 1024))                # w1B k0-1 (~13.3)
            nc.sync.dma_start(*icol(A2 + 1024, A2 + 2048))           # w1B k2-3 (~15)
            if P1 > P0:
                mid_c1 = min(P0 + 1024, P1)
                nc.scalar.dma_start(*icol(P0, mid_c1))               # st_c1 k0-1 (~16.8)
                if P1 > mid_c1:
                    mid_c2 = min(P0 + 2048, P1)
                    nc.sync.dma_start(*icol(mid_c1, mid_c2))         # st_c1 k2-3 (~18.5)
                    if P1 > mid_c2:
                        nc.sync.dma_start(*icol(mid_c2, P1))         # st_tail (~24.5)
            # scalar's desc burst ENDS here (6 descs, ~11.3us): its 4-deep
            # HWDGE window paces descs by DMA completions, so any more
            # would collide with its relu1 ACT duty (~13us) and back up the
            # PSUM pool (measured 1.6us PE stall).  All remaining inputs
            # ride sync, which has no elementwise work.
            nc.sync.dma_start(w2qt[:, :, :], w2q_ext[:])             # w2 fp8 (~25.6)
            nc.sync.dma_start(*icol(P1, half2))                      # w2 f16 k2-4 (~25.8)
            nc.sync.dma_start(*icol(half2, P2))                      # w2 f16 k5-7 (~27)
            nc.sync.dma_start(*icol(P2, IMG_W))                      # w3 (~37.8+)

            def l1_mm(ps, p, k, c0, cw):
                w0 = w1_col(k, p)
                s0 = st_col(k, c0, cw)
                nc.tensor.matmul(
                    ps[:],
                    img[:, w0 : w0 + 128],
                    img[:, s0 : s0 + cw],
                    start=(k == 0),
                    stop=(k == KD - 1),
                )

            DR = mybir.MatmulPerfMode.DoubleRow

            def l2_dr(ps, m, kk, rhs, start, stop=False):
                # fp8 DoubleRow: one matmul contracts k-tiles kk and kk+1
                nc.tensor.matmul(
                    ps[:],
                    w2qt[:, kk : kk + 2, m * 128 : (m + 1) * 128],
                    rhs,
                    start=start,
                    stop=stop,
                    perf_mode=DR,
                    skip_group_check=True,
                )

            def l2_mm(ps, m, k, sl):
                w0 = P1 + (k - 2) * H2 + m * 128
                nc.tensor.matmul(
                    ps[:],
                    img[:, w0 : w0 + 128],
                    h1[k][:, sl],
                    start=False,
                    stop=(k == K1 - 1),
                    skip_group_check=True,
                )

            # relu split: image positions 0-3 (first wave) on vector,
            # positions 4-7 (second wave) on scalar ACT.  Logical m0-1 are
            # fp8: out = Relu(16*ps + 16*b1) cast to e4m3 (bias pre-scaled
            # host-side).  Tail-chunk m2-7 additionally write scale-1 e4m3.
            def relu1(ps, p, sl, in_c2=False):
                lm = _MPERM[p]
                if lm < 2:
                    nc.scalar.activation(
                        h1q[:, lm, sl], ps[:], Relu, bias=bt[:, p : p + 1], scale=16.0
                    )
                elif in_c2:
                    if p % 2 == 0:
                        nc.vector.tensor_scalar(
                            h1q2[:, lm - 2, :], ps[:], bt[:, p : p + 1], 0.0, add, amax
                        )
                    else:
                        nc.scalar.activation(
                            h1q2[:, lm - 2, :], ps[:], Relu, bias=bt[:, p : p + 1]
                        )
                elif p % 2 == 0:
                    nc.vector.tensor_scalar(
                        h1[lm][:, sl], ps[:], bt[:, p : p + 1], 0.0, add, amax
                    )
                else:
                    nc.scalar.activation(
                        h1[lm][:, sl], ps[:], Relu, bias=bt[:, p : p + 1]
                    )

            def relu2(ps, m, sl):
                if m < M2 // 2:
                    nc.vector.tensor_scalar(
                        hf[m][:, sl], ps[:], bt[:, M1 + m : M1 + m + 1], 0.0, add, amax
                    )
                else:
                    nc.scalar.activation(
                        hf[m][:, sl], ps[:], Relu, bias=bt[:, M1 + m : M1 + m + 1]
                    )

            # Out-DMA queue split: the first four pieces ride scalar (slack
            # there once its input descs end ~11.3us), the last four ride
            # sync — keeps sync's total DMA count at 16 (12 in + 4 out) so
            # the closely-spaced FINAL out-DMAs never wait on a 4-deep
            # sem-window slot still held by a recent DMA's ~2.6us
            # completion (measured +2.8us when violated), and the last
            # group's DMA issues promptly after its tanh.
            out_dma_n = [0]

            def l3_piece(j, g0, g1):
                # L3 for columns [g0, g1) of group j (W3 slot j), split into
                # <=512-col PSUM banks; tanh descales the 2^12 PSUM scale.
                for p0 in range(g0, g1, 512):
                    p1 = min(p0 + 512, g1)
                    ps = pp3.tile([A, p1 - p0], f32, name="ps3", tag="ps3")
                    for k in range(K2):
                        w0 = P2 + j * K2 * A + k * A
                        nc.tensor.matmul(
                            ps[:],
                            img[:, w0 : w0 + A],
                            hf[k][:, p0:p1],
                            start=(k == 0),
                            stop=(k == K2 - 1),
                        )
                    nc.scalar.activation(
                        osb[:, p0:p1], ps[:], Tanh,
                        bias=bt[:, M1 + M2 + j : M1 + M2 + j + 1],
                        scale=1.0 / _SF,
                    )
                    eng = nc.scalar if out_dma_n[0] < 4 else nc.sync
                    out_dma_n[0] += 1
                    eng.dma_start(out_ext[:, p0:p1], osb[:, p0:p1])

            # Phase 1: L1 for ALL chunks (needs only st+w1) so w2/w3 stream
            # in behind the compute.  Wave order tracks DMA arrival.  The
            # small tail chunk runs last, per (m, k).
            big = [c for c in chunks if c[1] == 512]
            small = [c for c in chunks if c[1] < 512]
            for cc0, ccw in big:
                for half in (range(0, 4), range(4, M1)):
                    pss = [pp.tile([128, ccw], f32, name="ps", tag="ps") for _ in half]
                    for k in range(KD):
                        for i, p in enumerate(half):
                            l1_mm(pss[i], p, k, cc0, ccw)
                    for i, p in enumerate(half):
                        relu1(pss[i], p, slice(cc0, cc0 + ccw))
            for p in range(M1):
                for cc0, ccw in small:
                    ps = pp.tile([128, ccw], f32, name="ps", tag="ps")
                    for k in range(KD):
                        l1_mm(ps, p, k, cc0, ccw)
                    relu1(ps, p, slice(cc0, cc0 + ccw), in_c2=True)

            # Phase 2+3: L2 per chunk, then L3 for fully-covered groups.
            # Big chunks: DR(k0-1) + 6 fp16 passes.  Tail chunk: 4 DRs.
            # (Splitting the boundary-crossing group's L3 at the tail
            # boundary REGRESSES: post-tail critical path is 8 issue-bound
            # passes + tanh + DMA regardless of piece width, so a split
            # only adds passes.)
            done_j = 0
            for ci, (c0, cw) in enumerate(chunks):
                sl = slice(c0, c0 + cw)
                for m in range(M2):
                    ps = pp.tile([128, cw], f32, name="ps", tag="ps")
                    if cw < 512 and c2w:
                        l2_dr(ps, m, 0, h1q[:, :, sl], True)
                        l2_dr(ps, m, 2, h1q2[:, 0:2, :], False)
                        l2_dr(ps, m, 4, h1q2[:, 2:4, :], False)
                        l2_dr(ps, m, 6, h1q2[:, 4:6, :], False, stop=True)
                    else:
                        l2_dr(ps, m, 0, h1q[:, :, sl], True)
                        for k in range(2, K1):
                            l2_mm(ps, m, k, sl)
                    relu2(ps, m, sl)
                lim = c0 + cw
                while done_j < NG and starts[done_j + 1] <= lim:
                    l3_piece(done_j, int(starts[done_j]), int(starts[done_j + 1]))
                    done_j += 1
            assert done_j == NG

    nc.finalize()
    return nc


def _kmajor(w, K):
    """[K*128, F] -> [128, K*F] with col = k*F + f."""
    F = w.shape[1]
    return np.ascontiguousarray(w.reshape(K, 128, F).transpose(1, 0, 2).reshape(128, K * F))


def _prepare(state, idx, W1, b1, W2, b2, W3, b3):
    state = np.ascontiguousarray(np.asarray(state, dtype=np.float32))
    idx = np.asarray(idx)
    W1 = np.asarray(W1, dtype=np.float32)
    b1 = np.asarray(b1, dtype=np.float32)
    W2 = np.asarray(W2, dtype=np.float32)
    b2 = np.asarray(b2, dtype=np.float32)
    W3 = np.asarray(W3, dtype=np.float32)
    b3 = np.asarray(b3, dtype=np.float32)

    B, D = state.shape
    G, _, H1 = W1.shape
    H2 = W2.shape[1]
    A = W3.shape[2]
    KD, K1, K2 = D // 128, H1 // 128, H2 // 128
    M1, M2 = H1 // 128, H2 // 128
    assert G == _NCORES, f"expert-parallel plan assumes {_NCORES} games, got {G}"

    perm, core_groups, M, starts, N = _make_plan(idx, G)
    NG = len(M)
    chunks = _chunks_of(N)

    key = (D, H1, H2, A, NG, tuple(int(x) for x in starts), N)
    if key not in _graph_cache:
        _graph_cache[key] = _build_graph(D, H1, H2, A, NG, starts, N)
    nc = _graph_cache[key]

    # fused image column offsets (must mirror _build_graph)
    c0w0 = chunks[0][1]
    A1 = KD * c0w0
    A2 = A1 + KD * 512
    P0 = A2 + KD * 512
    P1 = P0 + KD * (N - c0w0)
    P2 = P1 + (K1 - 2) * H2
    IMG_W = P2 + NG * K2 * A

    # L2 k0-1 in fp8 at x256 (pairs h1q x16), k2-7 at x4096 (tail h1q2 x1);
    # fp16 w2 k2-7 pre-scaled x2^12 so all PSUM partials share one scale;
    # W3 ships UNSCALED fp16 (tanh ACT descales by 2^-12).
    w2q_f = np.concatenate([W2[:256] * _SW0, W2[256:] * _SW1])
    w2q = np.clip(w2q_f, -240.0, 240.0).astype(_FP8)
    w2q = np.ascontiguousarray(
        w2q.reshape(K1, 128, H2).transpose(1, 0, 2).reshape(128, K1 * H2)
    )
    w2_h = _kmajor((W2[256:] * _SF).astype(_FP16), K1 - 2)
    b2_col = (b2 * _SF).reshape(M2, 128).T.astype(np.float32)

    in_maps = []
    scatters = []  # per core: list of (sorted_positions, col_start)
    for c in range(G):
        sT = np.zeros((D, N), dtype=_FP16)
        img = np.zeros((128, IMG_W), dtype=_FP16)
        bs = np.zeros((128, M1 + M2 + NG), dtype=np.float32)
        bs[:, :M1] = b1[c].reshape(M1, 128)[list(_MPERM)].T
        for p_, lm_ in enumerate(_MPERM):   # relu1 logical m0-1: Relu(16*ps+16*b1)
            if lm_ < 2:
                bs[:, p_] *= _SH
        bs[:, M1 : M1 + M2] = b2_col
        sc = []
        for j, (head, pos) in enumerate(core_groups[c]):
            s0 = int(starts[j])
            if len(pos):
                sT[:, s0 : s0 + len(pos)] = state[perm[pos]].T.astype(_FP16)
                sc.append((pos, s0))
            img[:, P2 + j * K2 * A : P2 + (j + 1) * K2 * A] = (
                W3[head].astype(_FP16)
                .reshape(K2, 128, A).transpose(1, 0, 2).reshape(128, K2 * A)
            )
            bs[:, M1 + M2 + j] = b3[head]
        w1p = W1[c].reshape(D, M1, 128)[:, list(_MPERM), :].reshape(D, H1)
        w1_h = _kmajor(w1p.astype(_FP16), KD)    # [128, KD*H1], col k*H1+m
        for k in range(KD):
            img[:, k * c0w0 : (k + 1) * c0w0] = sT[k * 128 : (k + 1) * 128, 0:c0w0]
            # w1 halves: positions 0-3 at A1 (k-major), 4-7 at A2
            img[:, A1 + k * 512 : A1 + (k + 1) * 512] = w1_h[:, k * H1 : k * H1 + 512]
            img[:, A2 + k * 512 : A2 + (k + 1) * 512] = w1_h[:, k * H1 + 512 : (k + 1) * H1]
        for c0, cw in chunks[1:]:
            img[:, P0 + (c0 - c0w0) * KD : P0 + (c0 - c0w0 + cw) * KD] = (
                sT[:, c0 : c0 + cw].reshape(KD, 128, cw).transpose(1, 0, 2).reshape(128, KD * cw)
            )
        img[:, P1:P2] = w2_h
        in_maps.append({"img": img, "bs": bs, "w2q": w2q})
        scatters.append(sc)
    return nc, in_maps, scatters, (B, A)


def _run(state, idx, W1, b1, W2, b2, W3, b3, trace=False, trace_kwargs=None):
    from concourse.bass_utils import run_bass_kernel_spmd

    nc, in_maps, scatters, (B, A) = _prepare(state, idx, W1, b1, W2, b2, W3, b3)
    res = run_bass_kernel_spmd(
        nc,
        in_maps,
        core_ids=list(range(_NCORES)),
        trace=trace,
        **(trace_kwargs or {}),
    )
    out = np.zeros((B, A), dtype=np.float32)
    for c in range(len(scatters)):
        o = np.asarray(res.results[c]["out"], dtype=np.float32)  # [A, N]
        for pos, s0 in scatters[c]:
            out[pos] = o[:, s0 : s0 + len(pos)].T
    return out, res


def kernel(**inputs) -> np.ndarray:
    out, _ = _run(**inputs)
    return out
